# revision 9
# baseline (speedup 1.0000x reference)
"""VQ codebook (EuclCodebook) Trainium2 Bass kernel.

Data-parallel over 8 NeuronCores: z [32,1024,256] is sharded along batch
(4 batches = 4096 tokens per core); the codebook [8192,256] is replicated.

Per core:
  scores[t, k] = fl(fl(2 * (z_t . c_k)) - zsq_t)   (== -d[t,k] of the
  reference distance matrix bit-for-bit: the reference's csq term is
  entirely absorbed by fp32 rounding since zsq ~ 256 >> csq ~ 1e-9)
  idx = argmax_k scores (first occurrence on ties == jnp.argmin semantics)
  z_q = codebook[idx]  (HBM row gather)
  t = z_q - z; z_q_st = z + t; res = z - z_q_st  (exact fp32 elementwise,
  matches the reference's straight-through ops bitwise)
  loss partials = per-token-tile sums of t^2; combined on host.

The dominant work is the fp32 (32768x256)@(256x8192) distance matmul on
the PE array. fp32 (not bf16/fp32r) is required: the argmin compares fp32
distance values quantized at ulp(zsq)~3e-5, and lower-precision matmul
flips hundreds of near-tie argmins vs the reference.
"""

import sys

if "/opt/trn_rl_repo" not in sys.path:
    sys.path.insert(0, "/opt/trn_rl_repo")

import numpy as np
from concourse import bacc, mybir, tile, masks
from concourse.bass_utils import run_bass_kernel_spmd

N_CORES = 8
B, L, E, K = 32, 1024, 256, 8192
T = B * L // N_CORES  # tokens per core = 4096
TT = T // 128         # token tiles per core = 32
NCT = K // 512        # code tiles = 16
F32 = mybir.dt.float32
I32 = mybir.dt.int32
I16 = mybir.dt.int16
U32 = mybir.dt.uint32


def build_nc():
    nc = bacc.Bacc("TRN2", target_bir_lowering=False, debug=False)

    z_d = nc.dram_tensor("z", [T, E], F32, kind="ExternalInput").ap()
    cb_d = nc.dram_tensor("codebook", [K, E], F32, kind="ExternalInput").ap()
    zq_d = nc.dram_tensor("zq_st", [T, E], F32, kind="ExternalOutput").ap()
    res_d = nc.dram_tensor("res", [T, E], F32, kind="ExternalOutput").ap()
    idx_d = nc.dram_tensor("idx", [TT, 128], I32, kind="ExternalOutput").ap()
    ls_d = nc.dram_tensor("lsum", [128, TT], F32, kind="ExternalOutput").ap()
    # int16 index scratch for the dma_gather wrapped layout round-trip:
    # token t = j*128 + s*16 + a  <->  [j, s, a]
    i16_d = nc.dram_tensor("i16scratch", [TT, 8, 16], I16).ap()

    with tile.TileContext(nc) as tc:
        with (
            tc.tile_pool(name="const", bufs=1) as const_pool,
            tc.tile_pool(name="cbt", bufs=1) as cbt_pool,
            tc.tile_pool(name="cbld", bufs=3) as cbld_pool,
            tc.tile_pool(name="zld", bufs=6) as zld_pool,
            tc.tile_pool(name="zt", bufs=3) as zt_pool,
            tc.tile_pool(name="sq", bufs=2) as sq_pool,
            tc.tile_pool(name="zsq", bufs=4) as zsq_pool,
            tc.tile_pool(name="scores", bufs=2) as scores_pool,
            tc.tile_pool(name="mx", bufs=4) as mx_pool,
            tc.tile_pool(name="i16", bufs=4) as i16_pool,
            tc.tile_pool(name="wrap", bufs=4) as wrap_pool,
            tc.tile_pool(name="zq", bufs=3) as zq_pool,
            tc.tile_pool(name="ep", bufs=3) as ep_pool,
            tc.tile_pool(name="ls", bufs=1) as ls_pool,
            tc.tile_pool(name="tp_psum", bufs=2, space="PSUM") as tp_psum,
            tc.tile_pool(name="mm_psum", bufs=4, space="PSUM") as mm_psum,
        ):
            ident = const_pool.tile([128, 128], F32)
            masks.make_identity(nc, ident[:])

            # --- one-time: transpose codebook into cbT[e_chunk][k] ---
            # split into quarters so early matmuls don't wait on the full
            # 8 MB codebook transpose
            NQ = 4
            KQ = K // NQ  # 2048 codes per quarter
            cbt_q = [cbt_pool.tile([128, 2, KQ], F32, tag=f"cbt{q}", name=f"cbt{q}") for q in range(NQ)]
            for q in range(NQ):
                for kt in range(KQ // 128):
                    cb_tile = cbld_pool.tile([128, E], F32, tag="cbld")
                    kg = q * KQ + kt * 128
                    nc.sync.dma_start(out=cb_tile[:], in_=cb_d[kg:kg + 128, :])
                    for ch in range(2):
                        tp = tp_psum.tile([128, 128], F32, tag="tp")
                        nc.tensor.transpose(tp[:], cb_tile[:, ch * 128:(ch + 1) * 128], ident[:])
                        nc.vector.tensor_copy(cbt_q[q][:, ch, kt * 128:(kt + 1) * 128], tp[:])

            lsum = ls_pool.tile([128, TT], F32)

            for j in range(TT):
                # load z tile [128 tokens, 256]
                z_tile = zld_pool.tile([128, E], F32, tag="z")
                nc.sync.dma_start(out=z_tile[:], in_=z_d[j * 128:(j + 1) * 128, :])

                # zsq per token (ACT square with row-sum accumulator)
                sq_scr = sq_pool.tile([128, E], F32, tag="sq")
                zsq = zsq_pool.tile([128, 1], F32, tag="zsq")
                nc.scalar.activation(
                    sq_scr[:], z_tile[:], mybir.ActivationFunctionType.Square,
                    accum_out=zsq[:],
                )
                negzsq = zsq_pool.tile([128, 1], F32, tag="negzsq")
                nc.gpsimd.tensor_scalar_mul(negzsq[:], zsq[:], -1.0)

                # transpose z tile -> zT [e_p, chunk, token]
                zT = zt_pool.tile([128, 2, 128], F32, tag="zt")
                for ch in range(2):
                    tp = tp_psum.tile([128, 128], F32, tag="tp")
                    nc.tensor.transpose(tp[:], z_tile[:, ch * 128:(ch + 1) * 128], ident[:])
                    nc.vector.tensor_copy(zT[:, ch, :], tp[:])

                # distance matmuls + fused (2*zc - zsq) epilogue into scores.
                # 4 PSUM banks per weight load: same stationary zT chunk for 4
                # consecutive matmuls (lets codegen skip redundant LDWEIGHTS).
                scores = scores_pool.tile([128, K], F32, tag="scores")
                for g in range(NCT // 4):
                    pss = [mm_psum.tile([128, 512], F32, tag="mm", name=f"mm{j}_{g}_{i}") for i in range(4)]
                    for ch in range(2):
                        for i in range(4):
                            ct = g * 4 + i
                            nc.tensor.matmul(
                                pss[i][:], zT[:, ch, :],
                                cbt_q[ct // 4][:, ch, (ct % 4) * 512:(ct % 4 + 1) * 512],
                                start=(ch == 0), stop=(ch == 1),
                            )
                    for i in range(4):
                        ct = g * 4 + i
                        nc.scalar.activation(
                            scores[:, ct * 512:(ct + 1) * 512], pss[i][:],
                            mybir.ActivationFunctionType.Identity,
                            scale=2.0, bias=negzsq[:],
                        )

                # argmax over all 8192 codes (first occurrence on ties)
                mx = mx_pool.tile([128, 8], F32, tag="mx")
                mi = mx_pool.tile([128, 8], U32, tag="mi")
                nc.vector.max(mx[:], scores[:])
                nc.vector.max_index(mi[:], mx[:], scores[:])

                # int32 idx output
                nc.sync.dma_start(out=idx_d[j], in_=mi[:].bitcast(I32)[:, 0:1])

                # int16 index -> DRAM -> wrapped [16-partition] layout for gather
                i16 = i16_pool.tile([128, 1], I16, tag="i16")
                nc.gpsimd.tensor_copy(i16[:], mi[:].bitcast(I16)[:, 0:1])
                nc.sync.dma_start(out=i16_d[j], in_=i16[:])
                wrap = wrap_pool.tile([128, 8], I16, tag="wrap")
                for g in range(8):
                    nc.sync.dma_start(
                        out=wrap[g * 16:(g + 1) * 16, :],
                        in_=i16_d[j].rearrange("s a -> a s"),
                    )
                zq = zq_pool.tile([128, 1, E], F32, tag="zq")
                nc.gpsimd.dma_gather(
                    out_ap=zq[:], in_ap=cb_d[:, :], idxs_ap=wrap[:],
                    num_idxs=128, num_idxs_reg=128, elem_size=E,
                )

                # straight-through epilogue (exact fp32, matches reference ops)
                # on GpSimd so the gather round-trip never blocks the DVE FIFO
                tdiff = ep_pool.tile([128, E], F32, tag="td")
                nc.gpsimd.tensor_sub(tdiff[:], zq[:, 0, :], z_tile[:])
                sq2 = sq_pool.tile([128, E], F32, tag="sq")
                nc.scalar.activation(
                    sq2[:], tdiff[:], mybir.ActivationFunctionType.Square,
                    accum_out=lsum[:, j:j + 1],
                )
                zqst = ep_pool.tile([128, E], F32, tag="zqst")
                nc.gpsimd.tensor_add(zqst[:], z_tile[:], tdiff[:])
                resi = ep_pool.tile([128, E], F32, tag="resi")
                nc.gpsimd.tensor_sub(resi[:], z_tile[:], zqst[:])
                nc.sync.dma_start(out=zq_d[j * 128:(j + 1) * 128, :], in_=zqst[:])
                nc.sync.dma_start(out=res_d[j * 128:(j + 1) * 128, :], in_=resi[:])

            nc.sync.dma_start(out=ls_d[:, :], in_=lsum[:])

    nc.compile()
    return nc


_NC_CACHE = []
TRACE = False  # set True (before first kernel() call) to capture an NTFF profile


def _get_nc():
    if not _NC_CACHE:
        _NC_CACHE.append(build_nc())
    return _NC_CACHE[0]


def kernel(z, codebook, _results_hook=None):
    z = np.ascontiguousarray(np.asarray(z), dtype=np.float32)
    codebook = np.ascontiguousarray(np.asarray(codebook), dtype=np.float32)
    zf = z.reshape(-1, E)
    nc = _get_nc()
    in_maps = [
        {"z": zf[i * T:(i + 1) * T], "codebook": codebook} for i in range(N_CORES)
    ]
    r = run_bass_kernel_spmd(nc, in_maps, list(range(N_CORES)), trace=TRACE)
    if _results_hook is not None:
        _results_hook(r)
    res_maps = r.results
    zq_st = np.concatenate([res_maps[i]["zq_st"] for i in range(N_CORES)], axis=0)
    res = np.concatenate([res_maps[i]["res"] for i in range(N_CORES)], axis=0)
    idx = np.concatenate(
        [res_maps[i]["idx"].reshape(-1) for i in range(N_CORES)], axis=0
    )
    total_sq = np.float64(0.0)
    for i in range(N_CORES):
        total_sq += np.sum(res_maps[i]["lsum"].astype(np.float64))
    m = np.float32(total_sq / (B * L * E))
    loss = np.float32(m + m)
    return (
        zq_st.reshape(B, L, E),
        idx.reshape(B, L).astype(np.int32),
        loss,
        res.reshape(B, L, E),
    )


# revision 10
# speedup vs baseline: 1.3915x; 1.3915x over previous
"""VQ codebook (EuclCodebook) Trainium2 Bass kernel.

Data-parallel over 8 NeuronCores: z [32,1024,256] is sharded along batch
(4 batches = 4096 tokens per core); the codebook [8192,256] is replicated.

Per core:
  scores[t, k] = fl(fl(2 * (z_t . c_k)) - zsq_t)   (== -d[t,k] of the
  reference distance matrix bit-for-bit: the reference's csq term is
  entirely absorbed by fp32 rounding since zsq ~ 256 >> csq ~ 1e-9)
  idx = argmax_k scores (first occurrence on ties == jnp.argmin semantics)
  z_q = codebook[idx]  (HBM row gather)
  t = z_q - z; z_q_st = z + t; res = z - z_q_st  (exact fp32 elementwise,
  matches the reference's straight-through ops bitwise)
  loss partials = per-token-tile sums of t^2; combined on host.

The dominant work is the fp32 (32768x256)@(256x8192) distance matmul on
the PE array. fp32 (not bf16/fp32r) is required: the argmin compares fp32
distance values quantized at ulp(zsq)~3e-5, and lower-precision matmul
flips hundreds of near-tie argmins vs the reference.
"""

import sys

if "/opt/trn_rl_repo" not in sys.path:
    sys.path.insert(0, "/opt/trn_rl_repo")

import numpy as np
from concourse import bacc, mybir, tile, masks
from concourse.bass_utils import run_bass_kernel_spmd

N_CORES = 8
B, L, E, K = 32, 1024, 256, 8192
T = B * L // N_CORES  # tokens per core = 4096
TT = T // 128         # token tiles per core = 32
NCT = K // 512        # code tiles = 16
F32 = mybir.dt.float32
I32 = mybir.dt.int32
I16 = mybir.dt.int16
U32 = mybir.dt.uint32


def build_nc():
    nc = bacc.Bacc("TRN2", target_bir_lowering=False, debug=False)

    z_d = nc.dram_tensor("z", [T, E], F32, kind="ExternalInput").ap()
    cb_d = nc.dram_tensor("codebook", [K, E], F32, kind="ExternalInput").ap()
    zq_d = nc.dram_tensor("zq_st", [T, E], F32, kind="ExternalOutput").ap()
    res_d = nc.dram_tensor("res", [T, E], F32, kind="ExternalOutput").ap()
    idx_d = nc.dram_tensor("idx", [TT, 128], I32, kind="ExternalOutput").ap()
    ls_d = nc.dram_tensor("lsum", [128, TT], F32, kind="ExternalOutput").ap()
    # int16 index scratch for the dma_gather wrapped layout round-trip:
    # token t = j*128 + s*16 + a  <->  [j, s, a]
    i16_d = nc.dram_tensor("i16scratch", [TT, 8, 16], I16).ap()

    with tile.TileContext(nc) as tc:
        with (
            tc.tile_pool(name="const", bufs=1) as const_pool,
            tc.tile_pool(name="cbt", bufs=1) as cbt_pool,
            tc.tile_pool(name="cbld", bufs=3) as cbld_pool,
            tc.tile_pool(name="zld", bufs=6) as zld_pool,
            tc.tile_pool(name="zt", bufs=3) as zt_pool,
            tc.tile_pool(name="sq", bufs=2) as sq_pool,
            tc.tile_pool(name="zsq", bufs=4) as zsq_pool,
            tc.tile_pool(name="scores", bufs=2) as scores_pool,
            tc.tile_pool(name="mx", bufs=4) as mx_pool,
            tc.tile_pool(name="i16", bufs=4) as i16_pool,
            tc.tile_pool(name="wrap", bufs=4) as wrap_pool,
            tc.tile_pool(name="zq", bufs=3) as zq_pool,
            tc.tile_pool(name="ep", bufs=3) as ep_pool,
            tc.tile_pool(name="ls", bufs=1) as ls_pool,
            tc.tile_pool(name="tp_psum", bufs=2, space="PSUM") as tp_psum,
            tc.tile_pool(name="mm_psum", bufs=4, space="PSUM") as mm_psum,
        ):
            ident = const_pool.tile([128, 128], F32)
            masks.make_identity(nc, ident[:])

            # --- one-time: transpose codebook into cbT[e_chunk][k] ---
            # split into quarters so early matmuls don't wait on the full
            # 8 MB codebook transpose
            NQ = 4
            KQ = K // NQ  # 2048 codes per quarter
            cbt_q = [cbt_pool.tile([128, 2, KQ], F32, tag=f"cbt{q}", name=f"cbt{q}") for q in range(NQ)]
            for q in range(NQ):
                for kt in range(KQ // 128):
                    cb_tile = cbld_pool.tile([128, E], F32, tag="cbld")
                    kg = q * KQ + kt * 128
                    nc.sync.dma_start(out=cb_tile[:], in_=cb_d[kg:kg + 128, :])
                    for ch in range(2):
                        tp = tp_psum.tile([128, 128], F32, tag="tp")
                        nc.tensor.transpose(tp[:], cb_tile[:, ch * 128:(ch + 1) * 128], ident[:])
                        nc.vector.tensor_copy(cbt_q[q][:, ch, kt * 128:(kt + 1) * 128], tp[:])

            lsum = ls_pool.tile([128, TT], F32)

            z_tiles, mi_tiles = {}, {}

            def load_z(j):
                if j >= TT or j in z_tiles:
                    return
                zt_ = zld_pool.tile([128, E], F32, tag="z", name=f"z{j}")
                nc.sync.dma_start(out=zt_[:], in_=z_d[j * 128:(j + 1) * 128, :])
                z_tiles[j] = zt_

            def phase1(j):
                load_z(j + 2)
                z_tile = z_tiles[j]

                # zsq per token (ACT square with row-sum accumulator)
                sq_scr = sq_pool.tile([128, E], F32, tag="sq", name=f"sqs{j}")
                zsq = zsq_pool.tile([128, 1], F32, tag="zsq", name=f"zsq{j}")
                nc.scalar.activation(
                    sq_scr[:], z_tile[:], mybir.ActivationFunctionType.Square,
                    accum_out=zsq[:],
                )
                negzsq = zsq_pool.tile([128, 1], F32, tag="negzsq", name=f"nzsq{j}")
                nc.vector.tensor_scalar_mul(negzsq[:], zsq[:], -1.0)

                # transpose z tile -> zT [e_p, chunk, token]
                zT = zt_pool.tile([128, 2, 128], F32, tag="zt", name=f"zT{j}")
                for ch in range(2):
                    tp = tp_psum.tile([128, 128], F32, tag="tp", name=f"tp{j}_{ch}")
                    nc.tensor.transpose(tp[:], z_tile[:, ch * 128:(ch + 1) * 128], ident[:])
                    nc.vector.tensor_copy(zT[:, ch, :], tp[:])

                # distance matmuls + fused (2*zc - zsq) scores on ACT
                scores = scores_pool.tile([128, K], F32, tag="scores", name=f"sc{j}")
                for ct in range(NCT):
                    ps = mm_psum.tile([128, 512], F32, tag="mm", name=f"mm{j}_{ct}")
                    nc.tensor.matmul(
                        ps[:], zT[:, 0, :],
                        cbt_q[ct // 4][:, 0, (ct % 4) * 512:(ct % 4 + 1) * 512],
                        start=True, stop=False,
                    )
                    nc.tensor.matmul(
                        ps[:], zT[:, 1, :],
                        cbt_q[ct // 4][:, 1, (ct % 4) * 512:(ct % 4 + 1) * 512],
                        start=False, stop=True,
                    )
                    nc.scalar.activation(
                        scores[:, ct * 512:(ct + 1) * 512], ps[:],
                        mybir.ActivationFunctionType.Identity,
                        scale=2.0, bias=negzsq[:],
                    )

                # argmax over all 8192 codes (first occurrence on ties)
                mx = mx_pool.tile([128, 8], F32, tag="mx", name=f"mx{j}")
                mi = mx_pool.tile([128, 8], U32, tag="mi", name=f"mi{j}")
                nc.vector.max(mx[:], scores[:])
                nc.vector.max_index(mi[:], mx[:], scores[:])
                mi_tiles[j] = mi

                # int32 idx output
                nc.sync.dma_start(out=idx_d[j], in_=mi[:].bitcast(I32)[:, 0:1])

            def phase2(j):
                z_tile = z_tiles.pop(j)
                mi = mi_tiles.pop(j)
                # int16 index -> DRAM -> wrapped [16-partition] layout for gather
                # (all on the GpSimd queue: this chain has DRAM round-trip
                # latency and must not block the hot loop's queues)
                i16 = i16_pool.tile([128, 1], I16, tag="i16", name=f"i16{j}")
                nc.gpsimd.tensor_copy(i16[:], mi[:].bitcast(I16)[:, 0:1])
                nc.gpsimd.dma_start(out=i16_d[j], in_=i16[:])
                wrap = wrap_pool.tile([128, 8], I16, tag="wrap", name=f"wr{j}")
                for g in range(8):
                    nc.gpsimd.dma_start(
                        out=wrap[g * 16:(g + 1) * 16, :],
                        in_=i16_d[j].rearrange("s a -> a s"),
                    )
                zq = zq_pool.tile([128, 1, E], F32, tag="zq", name=f"zq{j}")
                nc.gpsimd.dma_gather(
                    out_ap=zq[:], in_ap=cb_d[:, :], idxs_ap=wrap[:],
                    num_idxs=128, num_idxs_reg=128, elem_size=E,
                )

                # straight-through epilogue (exact fp32, matches reference ops)
                tdiff = ep_pool.tile([128, E], F32, tag="td", name=f"td{j}")
                nc.gpsimd.tensor_sub(tdiff[:], zq[:, 0, :], z_tile[:])
                sq2 = sq_pool.tile([128, E], F32, tag="sq", name=f"sql{j}")
                nc.scalar.activation(
                    sq2[:], tdiff[:], mybir.ActivationFunctionType.Square,
                    accum_out=lsum[:, j:j + 1],
                )
                zqst = ep_pool.tile([128, E], F32, tag="zqst", name=f"zqst{j}")
                nc.gpsimd.tensor_add(zqst[:], z_tile[:], tdiff[:])
                resi = ep_pool.tile([128, E], F32, tag="resi", name=f"resi{j}")
                nc.gpsimd.tensor_sub(resi[:], z_tile[:], zqst[:])
                nc.sync.dma_start(out=zq_d[j * 128:(j + 1) * 128, :], in_=zqst[:])
                nc.sync.dma_start(out=res_d[j * 128:(j + 1) * 128, :], in_=resi[:])

            load_z(0)
            load_z(1)
            for step in range(TT + 2):
                if step < TT:
                    phase1(step)
                if step >= 2:
                    phase2(step - 2)

            nc.sync.dma_start(out=ls_d[:, :], in_=lsum[:])

    nc.compile()
    return nc


_NC_CACHE = []
TRACE = False  # set True (before first kernel() call) to capture an NTFF profile


def _get_nc():
    if not _NC_CACHE:
        _NC_CACHE.append(build_nc())
    return _NC_CACHE[0]


def kernel(z, codebook, _results_hook=None):
    z = np.ascontiguousarray(np.asarray(z), dtype=np.float32)
    codebook = np.ascontiguousarray(np.asarray(codebook), dtype=np.float32)
    zf = z.reshape(-1, E)
    nc = _get_nc()
    in_maps = [
        {"z": zf[i * T:(i + 1) * T], "codebook": codebook} for i in range(N_CORES)
    ]
    r = run_bass_kernel_spmd(nc, in_maps, list(range(N_CORES)), trace=TRACE)
    if _results_hook is not None:
        _results_hook(r)
    res_maps = r.results
    zq_st = np.concatenate([res_maps[i]["zq_st"] for i in range(N_CORES)], axis=0)
    res = np.concatenate([res_maps[i]["res"] for i in range(N_CORES)], axis=0)
    idx = np.concatenate(
        [res_maps[i]["idx"].reshape(-1) for i in range(N_CORES)], axis=0
    )
    total_sq = np.float64(0.0)
    for i in range(N_CORES):
        total_sq += np.sum(res_maps[i]["lsum"].astype(np.float64))
    m = np.float32(total_sq / (B * L * E))
    loss = np.float32(m + m)
    return (
        zq_st.reshape(B, L, E),
        idx.reshape(B, L).astype(np.int32),
        loss,
        res.reshape(B, L, E),
    )


# revision 14
# speedup vs baseline: 1.7393x; 1.2500x over previous
"""VQ codebook (EuclCodebook) Trainium2 Bass kernel.

Data-parallel over 8 NeuronCores: z [32,1024,256] is sharded along batch
(4 batches = 4096 tokens per core); the codebook [8192,256] is replicated.

Per core:
  scores[t, k] = fl(fl(2 * (z_t . c_k)) - zsq_t)   (== -d[t,k] of the
  reference distance matrix bit-for-bit: the reference's csq term is
  entirely absorbed by fp32 rounding since zsq ~ 256 >> csq ~ 1e-9)
  idx = argmax_k scores (first occurrence on ties == jnp.argmin semantics)
  z_q = codebook[idx]  (HBM row gather)
  t = z_q - z; z_q_st = z + t; res = z - z_q_st  (exact fp32 elementwise,
  matches the reference's straight-through ops bitwise)
  loss partials = per-token-tile sums of t^2; combined on host.

The dominant work is the fp32 (32768x256)@(256x8192) distance matmul on
the PE array. fp32 (not bf16/fp32r) is required: the argmin compares fp32
distance values quantized at ulp(zsq)~3e-5, and lower-precision matmul
flips hundreds of near-tie argmins vs the reference.
"""

import sys

if "/opt/trn_rl_repo" not in sys.path:
    sys.path.insert(0, "/opt/trn_rl_repo")

import numpy as np
from concourse import bacc, mybir, tile, masks
from concourse.bass_utils import run_bass_kernel_spmd

N_CORES = 8
B, L, E, K = 32, 1024, 256, 8192
T = B * L // N_CORES  # tokens per core = 4096
TT = T // 128         # token tiles per core = 32
NCT = K // 512        # code tiles = 16
F32 = mybir.dt.float32
I32 = mybir.dt.int32
I16 = mybir.dt.int16
U32 = mybir.dt.uint32


def build_nc():
    nc = bacc.Bacc("TRN2", target_bir_lowering=False, debug=False)

    z_d = nc.dram_tensor("z", [T, E], F32, kind="ExternalInput").ap()
    cb_d = nc.dram_tensor("codebook", [K, E], F32, kind="ExternalInput").ap()
    zq_d = nc.dram_tensor("zq_st", [T, E], F32, kind="ExternalOutput").ap()
    res_d = nc.dram_tensor("res", [T, E], F32, kind="ExternalOutput").ap()
    idx_d = nc.dram_tensor("idx", [TT, 128], I32, kind="ExternalOutput").ap()
    ls_d = nc.dram_tensor("lsum", [128, E], F32, kind="ExternalOutput").ap()
    # int16 index scratch for the dma_gather wrapped layout round-trip:
    # token t = j*128 + s*16 + a  <->  [j, s, a]
    i16_d = nc.dram_tensor("i16scratch", [TT, 8, 16], I16).ap()

    with tile.TileContext(nc) as tc:
        with (
            tc.tile_pool(name="const", bufs=1) as const_pool,
            tc.tile_pool(name="cbt", bufs=1) as cbt_pool,
            tc.tile_pool(name="cbld", bufs=3) as cbld_pool,
            tc.tile_pool(name="zld", bufs=6) as zld_pool,
            tc.tile_pool(name="zt", bufs=3) as zt_pool,
            tc.tile_pool(name="sq", bufs=2) as sq_pool,
            tc.tile_pool(name="zsq", bufs=4) as zsq_pool,
            tc.tile_pool(name="scores", bufs=2) as scores_pool,
            tc.tile_pool(name="mx", bufs=4) as mx_pool,
            tc.tile_pool(name="i16", bufs=4) as i16_pool,
            tc.tile_pool(name="wrap", bufs=4) as wrap_pool,
            tc.tile_pool(name="zq", bufs=3) as zq_pool,
            tc.tile_pool(name="ep", bufs=3) as ep_pool,
            tc.tile_pool(name="ls", bufs=1) as ls_pool,
            tc.tile_pool(name="tp_psum", bufs=2, space="PSUM") as tp_psum,
            tc.tile_pool(name="mm_psum", bufs=4, space="PSUM") as mm_psum,
        ):
            ident = const_pool.tile([128, 128], F32)
            masks.make_identity(nc, ident[:])

            # --- one-time: transpose codebook into cbT[e_chunk][k] ---
            # split into quarters so early matmuls don't wait on the full
            # 8 MB codebook transpose
            NQ = 4
            KQ = K // NQ  # 2048 codes per quarter
            cbt_q = [cbt_pool.tile([128, 2, KQ], F32, tag=f"cbt{q}", name=f"cbt{q}") for q in range(NQ)]
            for q in range(NQ):
                for kt in range(KQ // 128):
                    cb_tile = cbld_pool.tile([128, E], F32, tag="cbld")
                    kg = q * KQ + kt * 128
                    nc.sync.dma_start(out=cb_tile[:], in_=cb_d[kg:kg + 128, :])
                    for ch in range(2):
                        tp = tp_psum.tile([128, 128], F32, tag="tp")
                        nc.tensor.transpose(tp[:], cb_tile[:, ch * 128:(ch + 1) * 128], ident[:])
                        nc.vector.tensor_copy(cbt_q[q][:, ch, kt * 128:(kt + 1) * 128], tp[:])

            lacc = ls_pool.tile([128, E], F32)
            nc.gpsimd.memset(lacc[:], 0.0)

            z_tiles, mi_tiles = {}, {}

            def load_z(j):
                if j >= TT or j in z_tiles:
                    return
                zt_ = zld_pool.tile([128, E], F32, tag="z", name=f"z{j}")
                nc.sync.dma_start(out=zt_[:], in_=z_d[j * 128:(j + 1) * 128, :])
                z_tiles[j] = zt_

            def phase1(j):
                load_z(j + 2)
                z_tile = z_tiles[j]

                # zsq per token (ACT square with row-sum accumulator)
                sq_scr = sq_pool.tile([128, E], F32, tag="sq", name=f"sqs{j}")
                zsq = zsq_pool.tile([128, 1], F32, tag="zsq", name=f"zsq{j}")
                nc.scalar.activation(
                    sq_scr[:], z_tile[:], mybir.ActivationFunctionType.Square,
                    accum_out=zsq[:],
                )
                negzsq = zsq_pool.tile([128, 1], F32, tag="negzsq", name=f"nzsq{j}")
                nc.scalar.activation(
                    negzsq[:], zsq[:], mybir.ActivationFunctionType.Identity,
                    scale=-1.0,
                )

                # transpose z tile -> zT [e_p, chunk, token]
                zT = zt_pool.tile([128, 2, 128], F32, tag="zt", name=f"zT{j}")
                for ch in range(2):
                    tp = tp_psum.tile([128, 128], F32, tag="tp", name=f"tp{j}_{ch}")
                    nc.tensor.transpose(tp[:], z_tile[:, ch * 128:(ch + 1) * 128], ident[:])
                    nc.scalar.activation(
                        zT[:, ch, :], tp[:], mybir.ActivationFunctionType.Copy,
                    )

                # distance matmuls + fused (2*zc - zsq) scores on ACT
                scores = scores_pool.tile([128, K], F32, tag="scores", name=f"sc{j}")
                for ct in range(NCT):
                    ps = mm_psum.tile([128, 512], F32, tag="mm", name=f"mm{j}_{ct}")
                    nc.tensor.matmul(
                        ps[:], zT[:, 0, :],
                        cbt_q[ct // 4][:, 0, (ct % 4) * 512:(ct % 4 + 1) * 512],
                        start=True, stop=False,
                    )
                    nc.tensor.matmul(
                        ps[:], zT[:, 1, :],
                        cbt_q[ct // 4][:, 1, (ct % 4) * 512:(ct % 4 + 1) * 512],
                        start=False, stop=True,
                    )
                    nc.scalar.activation(
                        scores[:, ct * 512:(ct + 1) * 512], ps[:],
                        mybir.ActivationFunctionType.Identity,
                        scale=2.0, bias=negzsq[:],
                    )

                # argmax over all 8192 codes (first occurrence on ties)
                mx = mx_pool.tile([128, 8], F32, tag="mx", name=f"mx{j}")
                mi = mx_pool.tile([128, 8], U32, tag="mi", name=f"mi{j}")
                nc.vector.max(mx[:], scores[:])
                nc.vector.max_index(mi[:], mx[:], scores[:])
                mi_tiles[j] = mi

                # int32 idx output
                nc.sync.dma_start(out=idx_d[j], in_=mi[:].bitcast(I32)[:, 0:1])

            def phase2(j):
                z_tile = z_tiles.pop(j)
                mi = mi_tiles.pop(j)
                # int16 index -> DRAM -> wrapped [16-partition] layout for gather
                # (all on the GpSimd queue: this chain has DRAM round-trip
                # latency and must not block the hot loop's queues)
                i16 = i16_pool.tile([128, 1], I16, tag="i16", name=f"i16{j}")
                nc.gpsimd.tensor_copy(i16[:], mi[:].bitcast(I16)[:, 0:1])
                nc.gpsimd.dma_start(out=i16_d[j], in_=i16[:])
                wrap = wrap_pool.tile([128, 8], I16, tag="wrap", name=f"wr{j}")
                for g in range(8):
                    nc.gpsimd.dma_start(
                        out=wrap[g * 16:(g + 1) * 16, :],
                        in_=i16_d[j].rearrange("s a -> a s"),
                    )
                zq = zq_pool.tile([128, 1, E], F32, tag="zq", name=f"zq{j}")
                nc.gpsimd.dma_gather(
                    out_ap=zq[:], in_ap=cb_d[:, :], idxs_ap=wrap[:],
                    num_idxs=128, num_idxs_reg=128, elem_size=E,
                )

                # straight-through epilogue (exact fp32, matches reference ops)
                tdiff = ep_pool.tile([128, E], F32, tag="td", name=f"td{j}")
                nc.gpsimd.tensor_sub(tdiff[:], zq[:, 0, :], z_tile[:])
                sq2 = sq_pool.tile([128, E], F32, tag="sql", name=f"sql{j}")
                nc.gpsimd.tensor_mul(sq2[:], tdiff[:], tdiff[:])
                nc.gpsimd.tensor_add(lacc[:], lacc[:], sq2[:])
                zqst = ep_pool.tile([128, E], F32, tag="zqst", name=f"zqst{j}")
                nc.gpsimd.tensor_add(zqst[:], z_tile[:], tdiff[:])
                resi = ep_pool.tile([128, E], F32, tag="resi", name=f"resi{j}")
                nc.gpsimd.tensor_sub(resi[:], z_tile[:], zqst[:])
                nc.sync.dma_start(out=zq_d[j * 128:(j + 1) * 128, :], in_=zqst[:])
                nc.sync.dma_start(out=res_d[j * 128:(j + 1) * 128, :], in_=resi[:])

            load_z(0)
            load_z(1)
            for step in range(TT + 2):
                if step < TT:
                    phase1(step)
                if step >= 2:
                    phase2(step - 2)

            nc.sync.dma_start(out=ls_d[:, :], in_=lacc[:])

    nc.compile()
    return nc


_NC_CACHE = []
TRACE = False  # set True (before first kernel() call) to capture an NTFF profile


def _get_nc():
    if not _NC_CACHE:
        _NC_CACHE.append(build_nc())
    return _NC_CACHE[0]


def kernel(z, codebook, _results_hook=None):
    z = np.ascontiguousarray(np.asarray(z), dtype=np.float32)
    codebook = np.ascontiguousarray(np.asarray(codebook), dtype=np.float32)
    zf = z.reshape(-1, E)
    nc = _get_nc()
    in_maps = [
        {"z": zf[i * T:(i + 1) * T], "codebook": codebook} for i in range(N_CORES)
    ]
    r = run_bass_kernel_spmd(nc, in_maps, list(range(N_CORES)), trace=TRACE)
    if _results_hook is not None:
        _results_hook(r)
    res_maps = r.results
    zq_st = np.concatenate([res_maps[i]["zq_st"] for i in range(N_CORES)], axis=0)
    res = np.concatenate([res_maps[i]["res"] for i in range(N_CORES)], axis=0)
    idx = np.concatenate(
        [res_maps[i]["idx"].reshape(-1) for i in range(N_CORES)], axis=0
    )
    total_sq = np.float64(0.0)
    for i in range(N_CORES):
        total_sq += np.sum(res_maps[i]["lsum"].astype(np.float64))
    m = np.float32(total_sq / (B * L * E))
    loss = np.float32(m + m)
    return (
        zq_st.reshape(B, L, E),
        idx.reshape(B, L).astype(np.int32),
        loss,
        res.reshape(B, L, E),
    )


# revision 15
# speedup vs baseline: 1.7927x; 1.0307x over previous
"""VQ codebook (EuclCodebook) Trainium2 Bass kernel.

Data-parallel over 8 NeuronCores: z [32,1024,256] is sharded along batch
(4 batches = 4096 tokens per core); the codebook [8192,256] is replicated.

Per core:
  scores[t, k] = fl(fl(2 * (z_t . c_k)) - zsq_t)   (== -d[t,k] of the
  reference distance matrix bit-for-bit: the reference's csq term is
  entirely absorbed by fp32 rounding since zsq ~ 256 >> csq ~ 1e-9)
  idx = argmax_k scores (first occurrence on ties == jnp.argmin semantics)
  z_q = codebook[idx]  (HBM row gather)
  t = z_q - z; z_q_st = z + t; res = z - z_q_st  (exact fp32 elementwise,
  matches the reference's straight-through ops bitwise)
  loss partials = per-token-tile sums of t^2; combined on host.

The dominant work is the fp32 (32768x256)@(256x8192) distance matmul on
the PE array. fp32 (not bf16/fp32r) is required: the argmin compares fp32
distance values quantized at ulp(zsq)~3e-5, and lower-precision matmul
flips hundreds of near-tie argmins vs the reference.
"""

import sys

if "/opt/trn_rl_repo" not in sys.path:
    sys.path.insert(0, "/opt/trn_rl_repo")

import numpy as np
from concourse import bacc, mybir, tile, masks
from concourse.bass_utils import run_bass_kernel_spmd

N_CORES = 8
B, L, E, K = 32, 1024, 256, 8192
T = B * L // N_CORES  # tokens per core = 4096
TT = T // 128         # token tiles per core = 32
NCT = K // 512        # code tiles = 16
F32 = mybir.dt.float32
I32 = mybir.dt.int32
I16 = mybir.dt.int16
U32 = mybir.dt.uint32


def build_nc():
    nc = bacc.Bacc("TRN2", target_bir_lowering=False, debug=False)

    z_d = nc.dram_tensor("z", [T, E], F32, kind="ExternalInput").ap()
    cb_d = nc.dram_tensor("codebook", [K, E], F32, kind="ExternalInput").ap()
    zq_d = nc.dram_tensor("zq_st", [T, E], F32, kind="ExternalOutput").ap()
    res_d = nc.dram_tensor("res", [T, E], F32, kind="ExternalOutput").ap()
    idx_d = nc.dram_tensor("idx", [TT, 128], I32, kind="ExternalOutput").ap()
    ls_d = nc.dram_tensor("lsum", [128, E], F32, kind="ExternalOutput").ap()
    # int16 index scratch for the dma_gather wrapped layout round-trip:
    # token t = j*128 + s*16 + a  <->  [j, s, a]
    i16_d = nc.dram_tensor("i16scratch", [TT, 8, 16], I16).ap()

    with tile.TileContext(nc) as tc:
        with (
            tc.tile_pool(name="const", bufs=1) as const_pool,
            tc.tile_pool(name="cbt", bufs=1) as cbt_pool,
            tc.tile_pool(name="cbld", bufs=3) as cbld_pool,
            tc.tile_pool(name="zld", bufs=6) as zld_pool,
            tc.tile_pool(name="zt", bufs=3) as zt_pool,
            tc.tile_pool(name="sq", bufs=2) as sq_pool,
            tc.tile_pool(name="zsq", bufs=4) as zsq_pool,
            tc.tile_pool(name="scores", bufs=2) as scores_pool,
            tc.tile_pool(name="mx", bufs=4) as mx_pool,
            tc.tile_pool(name="i16", bufs=4) as i16_pool,
            tc.tile_pool(name="wrap", bufs=4) as wrap_pool,
            tc.tile_pool(name="zq", bufs=3) as zq_pool,
            tc.tile_pool(name="ep", bufs=3) as ep_pool,
            tc.tile_pool(name="ls", bufs=1) as ls_pool,
            tc.tile_pool(name="tp_psum", bufs=2, space="PSUM") as tp_psum,
            tc.tile_pool(name="mm_psum", bufs=4, space="PSUM") as mm_psum,
        ):
            ident = const_pool.tile([128, 128], F32)
            masks.make_identity(nc, ident[:])

            # --- one-time: transpose codebook into cbT[e_chunk][k] ---
            # split into quarters so early matmuls don't wait on the full
            # 8 MB codebook transpose
            NQ = 4
            KQ = K // NQ  # 2048 codes per quarter
            cbt_q = [cbt_pool.tile([128, 2, KQ], F32, tag=f"cbt{q}", name=f"cbt{q}") for q in range(NQ)]
            for q in range(NQ):
                for kt in range(KQ // 128):
                    cb_tile = cbld_pool.tile([128, E], F32, tag="cbld")
                    kg = q * KQ + kt * 128
                    nc.sync.dma_start(out=cb_tile[:], in_=cb_d[kg:kg + 128, :])
                    for ch in range(2):
                        tp = tp_psum.tile([128, 128], F32, tag="tp")
                        nc.tensor.transpose(tp[:], cb_tile[:, ch * 128:(ch + 1) * 128], ident[:])
                        nc.vector.tensor_copy(cbt_q[q][:, ch, kt * 128:(kt + 1) * 128], tp[:])

            lacc = ls_pool.tile([128, E], F32)
            nc.gpsimd.memset(lacc[:], 0.0)

            z_tiles, mi_tiles = {}, {}

            def load_z(j):
                if j >= TT or j in z_tiles:
                    return
                zt_ = zld_pool.tile([128, E], F32, tag="z", name=f"z{j}")
                nc.sync.dma_start(out=zt_[:], in_=z_d[j * 128:(j + 1) * 128, :])
                z_tiles[j] = zt_

            def phase1(j):
                load_z(j + 2)
                z_tile = z_tiles[j]

                # zsq per token (ACT square with row-sum accumulator)
                sq_scr = sq_pool.tile([128, E], F32, tag="sq", name=f"sqs{j}")
                zsq = zsq_pool.tile([128, 1], F32, tag="zsq", name=f"zsq{j}")
                nc.scalar.activation(
                    sq_scr[:], z_tile[:], mybir.ActivationFunctionType.Square,
                    accum_out=zsq[:],
                )
                negzsq = zsq_pool.tile([128, 1], F32, tag="negzsq", name=f"nzsq{j}")
                nc.scalar.activation(
                    negzsq[:], zsq[:], mybir.ActivationFunctionType.Identity,
                    scale=-1.0,
                )

                # transpose z tile -> zT [e_p, chunk, token]
                zT = zt_pool.tile([128, 2, 128], F32, tag="zt", name=f"zT{j}")
                for ch in range(2):
                    tp = tp_psum.tile([128, 128], F32, tag="tp", name=f"tp{j}_{ch}")
                    nc.tensor.transpose(tp[:], z_tile[:, ch * 128:(ch + 1) * 128], ident[:])
                    nc.scalar.activation(
                        zT[:, ch, :], tp[:], mybir.ActivationFunctionType.Copy,
                    )

                # distance matmuls + fused (2*zc - zsq) scores on ACT
                scores = scores_pool.tile([128, K], F32, tag="scores", name=f"sc{j}")
                for ct in range(NCT):
                    ps = mm_psum.tile([128, 512], F32, tag="mm", name=f"mm{j}_{ct}")
                    nc.tensor.matmul(
                        ps[:], zT[:, 0, :],
                        cbt_q[ct // 4][:, 0, (ct % 4) * 512:(ct % 4 + 1) * 512],
                        start=True, stop=False,
                    )
                    nc.tensor.matmul(
                        ps[:], zT[:, 1, :],
                        cbt_q[ct // 4][:, 1, (ct % 4) * 512:(ct % 4 + 1) * 512],
                        start=False, stop=True,
                    )
                    nc.scalar.activation(
                        scores[:, ct * 512:(ct + 1) * 512], ps[:],
                        mybir.ActivationFunctionType.Identity,
                        scale=2.0, bias=negzsq[:],
                    )

                # argmax over all 8192 codes (first occurrence on ties)
                mx = mx_pool.tile([128, 8], F32, tag="mx", name=f"mx{j}")
                mi = mx_pool.tile([128, 8], U32, tag="mi", name=f"mi{j}")
                nc.vector.max(mx[:], scores[:])
                nc.vector.max_index(mi[:], mx[:], scores[:])
                mi_tiles[j] = mi

            def phase2(j):
                z_tile = z_tiles.pop(j)
                mi = mi_tiles.pop(j)
                # int16 index -> DRAM -> wrapped [16-partition] layout for gather
                # (all on the GpSimd queue: this chain has DRAM round-trip
                # latency and must not block the hot loop's queues)
                i16 = i16_pool.tile([128, 1], I16, tag="i16", name=f"i16{j}")
                nc.gpsimd.tensor_copy(i16[:], mi[:].bitcast(I16)[:, 0:1])
                nc.gpsimd.dma_start(out=i16_d[j], in_=i16[:])
                wrap = wrap_pool.tile([128, 8], I16, tag="wrap", name=f"wr{j}")
                for g in range(8):
                    nc.gpsimd.dma_start(
                        out=wrap[g * 16:(g + 1) * 16, :],
                        in_=i16_d[j].rearrange("s a -> a s"),
                    )
                zq = zq_pool.tile([128, 1, E], F32, tag="zq", name=f"zq{j}")
                nc.gpsimd.dma_gather(
                    out_ap=zq[:], in_ap=cb_d[:, :], idxs_ap=wrap[:],
                    num_idxs=128, num_idxs_reg=128, elem_size=E,
                )

                # straight-through epilogue (exact fp32, matches reference ops)
                tdiff = ep_pool.tile([128, E], F32, tag="td", name=f"td{j}")
                nc.gpsimd.tensor_sub(tdiff[:], zq[:, 0, :], z_tile[:])
                sq2 = sq_pool.tile([128, E], F32, tag="sql", name=f"sql{j}")
                nc.gpsimd.tensor_mul(sq2[:], tdiff[:], tdiff[:])
                nc.gpsimd.tensor_add(lacc[:], lacc[:], sq2[:])
                zqst = ep_pool.tile([128, E], F32, tag="zqst", name=f"zqst{j}")
                nc.gpsimd.tensor_add(zqst[:], z_tile[:], tdiff[:])
                resi = ep_pool.tile([128, E], F32, tag="resi", name=f"resi{j}")
                nc.gpsimd.tensor_sub(resi[:], z_tile[:], zqst[:])
                nc.gpsimd.dma_start(out=zq_d[j * 128:(j + 1) * 128, :], in_=zqst[:])
                nc.gpsimd.dma_start(out=res_d[j * 128:(j + 1) * 128, :], in_=resi[:])
                nc.gpsimd.dma_start(out=idx_d[j], in_=mi[:].bitcast(I32)[:, 0:1])

            load_z(0)
            load_z(1)
            for step in range(TT + 2):
                if step < TT:
                    phase1(step)
                if step >= 2:
                    phase2(step - 2)

            nc.sync.dma_start(out=ls_d[:, :], in_=lacc[:])

    nc.compile()
    return nc


_NC_CACHE = []
TRACE = False  # set True (before first kernel() call) to capture an NTFF profile


def _get_nc():
    if not _NC_CACHE:
        _NC_CACHE.append(build_nc())
    return _NC_CACHE[0]


def kernel(z, codebook, _results_hook=None):
    z = np.ascontiguousarray(np.asarray(z), dtype=np.float32)
    codebook = np.ascontiguousarray(np.asarray(codebook), dtype=np.float32)
    zf = z.reshape(-1, E)
    nc = _get_nc()
    in_maps = [
        {"z": zf[i * T:(i + 1) * T], "codebook": codebook} for i in range(N_CORES)
    ]
    r = run_bass_kernel_spmd(nc, in_maps, list(range(N_CORES)), trace=TRACE)
    if _results_hook is not None:
        _results_hook(r)
    res_maps = r.results
    zq_st = np.concatenate([res_maps[i]["zq_st"] for i in range(N_CORES)], axis=0)
    res = np.concatenate([res_maps[i]["res"] for i in range(N_CORES)], axis=0)
    idx = np.concatenate(
        [res_maps[i]["idx"].reshape(-1) for i in range(N_CORES)], axis=0
    )
    total_sq = np.float64(0.0)
    for i in range(N_CORES):
        total_sq += np.sum(res_maps[i]["lsum"].astype(np.float64))
    m = np.float32(total_sq / (B * L * E))
    loss = np.float32(m + m)
    return (
        zq_st.reshape(B, L, E),
        idx.reshape(B, L).astype(np.int32),
        loss,
        res.reshape(B, L, E),
    )
